# revision 17
# baseline (speedup 1.0000x reference)
"""Trainium2 Bass kernel for nn_BoundaryPredictor2 (segment_reduce).

Data-parallel over batch B=8 across 8 NeuronCores; only the scalar
num_boundaries / total_positions are all-reduced.

Per-core algorithm (batch row h [L=2048, D=1024], fp32 throughout):
  M = Wq^T @ Wk                       (PE, [D, D])
  HT = h^T                            (PE transposes)
  UT[d, l] = sum_j M[j, d] HT[j, l]   (PE; = (h @ M)^T)
  praw[l]  = sum_d UT[d, l] HT[d, l+1]   (DVE mul fused w/ PSUM evac + PE ones-reduce)
  nrm2[l]  = sum_d HT[d, l]^2            (ACT square + PE ones-reduce)
  cos[l]   = praw[l-1] * rsqrt(nrm2[l-1] nrm2[l]);  cos[0] = -1
  hard     = (clip((1-cos)/2, eps, 1-eps) + clip(noise, eps, 1-eps)) > 1
             -- algebraically identical to sigmoid(logit(p)+logit(u)) > 1/2
  seg ids via cumsum of hard (trailing boundary forced at L-1), boundary
  positions e[j] scattered by rank (indirect DMA), block prefix sums of h
  (PE triangular matmuls + cross-block offsets), pooled[j] =
  (P[e_j] - P[e_{j-1}]) / (e_j - e_{j-1}) via indirect row gather + shifted
  diffs.  pooled_mask scattered likewise.  Scalars all-reduced; loss from a
  Stirling lgamma on-device.
"""

import numpy as np

L = 2048
D = 1024
P = 128
T = L // P           # 16 l-blocks of 128 (natural fold)
TF = L // P          # 16 cols in the (p-major) F2 fold: l = 16*p + t
NC = 8
EPS = 1.1920929e-07
LOG02 = float(np.log(np.float32(0.2)))
LOG08 = float(np.log1p(np.float32(-0.2)))
HALF_LN_2PI = float(np.float32(0.5 * np.log(2.0 * np.pi)))

_BUILT = None


def _consts():
    tri_incl = (np.arange(P)[:, None] <= np.arange(P)[None, :]).astype(np.float32)
    tri16s = (np.arange(16)[:, None] < np.arange(16)[None, :]).astype(np.float32)
    sel = np.zeros((P, 16, 16), np.float32)
    for t in range(16):
        sel[:, t, t] = 1.0
    selrow = np.zeros((16, 16, P), np.float32)
    for t in range(16):
        selrow[t, t, :] = 1.0
    ones = np.ones((P, P), np.float32)
    ident = np.eye(P, dtype=np.float32)
    liota = (np.arange(P)[:, None] * TF + np.arange(TF)[None, :]).astype(np.int32)
    efill = np.full((L + 1, 1), L - 1, np.int32)
    efill[0, 0] = -1
    liota_f = (np.arange(P)[:, None] * TF + np.arange(TF)[None, :]).astype(np.float32)
    lastm = np.zeros((P, TF), np.float32)
    lastm[P - 1, TF - 1] = 1.0
    return dict(tri_incl=tri_incl, tri16s=tri16s, sel=sel, selrow=selrow,
                ones=ones, ident=ident, liota=liota, liota_f=liota_f, efill=efill, lastm=lastm)


def _build():
    import concourse.bacc as bacc
    import concourse.mybir as mybir
    import concourse.tile as tile
    import concourse.bass as bass

    f32 = mybir.dt.float32
    i32 = mybir.dt.int32
    A = mybir.AluOpType
    AF = mybir.ActivationFunctionType

    nc = bacc.Bacc("TRN2", target_bir_lowering=False, debug=False, num_devices=NC)

    # ---- dram parameters ----
    h_d = nc.declare_dram_parameter("h", [L, D], f32, isOutput=False)
    wq_d = nc.declare_dram_parameter("Wq", [D, D], f32, isOutput=False)
    wk_d = nc.declare_dram_parameter("Wk", [D, D], f32, isOutput=False)
    mask_d = nc.declare_dram_parameter("mask_f2", [P, TF], i32, isOutput=False)
    noise_d = nc.declare_dram_parameter("noise_f2", [P, TF], f32, isOutput=False)
    tri_d = nc.declare_dram_parameter("tri_incl", [P, P], f32, isOutput=False)
    tri16_d = nc.declare_dram_parameter("tri16s", [16, 16], f32, isOutput=False)
    sel_d = nc.declare_dram_parameter("sel", [P, 16, 16], f32, isOutput=False)
    selrow_d = nc.declare_dram_parameter("selrow", [16, 16, P], f32, isOutput=False)
    ones_d = nc.declare_dram_parameter("ones", [P, P], f32, isOutput=False)
    ident_d = nc.declare_dram_parameter("ident", [P, P], f32, isOutput=False)
    liota_d = nc.declare_dram_parameter("liota", [P, TF], i32, isOutput=False)
    liotaf_d = nc.declare_dram_parameter("liota_f", [P, TF], f32, isOutput=False)
    efill_d = nc.declare_dram_parameter("efill", [L + 1, 1], i32, isOutput=False)
    lastm_d = nc.declare_dram_parameter("lastm", [P, TF], f32, isOutput=False)

    pooled_d = nc.declare_dram_parameter("pooled", [L, D], f32, isOutput=True)
    pmask_d = nc.declare_dram_parameter("pmask", [L, 1], i32, isOutput=True)
    loss_d = nc.declare_dram_parameter("loss", [1, 1], f32, isOutput=True)
    nb_d = nc.declare_dram_parameter("nb", [1, 1], f32, isOutput=True)
    tp_d = nc.declare_dram_parameter("tp", [1, 1], f32, isOutput=True)

    # ---- internal dram ----
    Pp = nc.dram_tensor("Pp", [L + 1, D], f32)           # prefix, row 0 = 0
    e_buf = nc.dram_tensor("e_buf", [L + 1, 1], i32)   # row 0 = -1 sentinel
    stg_dram = nc.dram_tensor("stg_dram", [1, L], f32)
    n2x_dram = nc.dram_tensor("n2x_dram", [1, L + 1], f32)
    cc_in = nc.dram_tensor("cc_in", [1, 2], f32)
    cc_out = nc.dram_tensor("cc_out", [1, 2], f32, addr_space="Shared")

    from contextlib import ExitStack

    with tile.TileContext(nc) as tc, ExitStack() as ctx:
        cp = ctx.enter_context(tc.tile_pool(name="consts", bufs=1))
        sp = ctx.enter_context(tc.tile_pool(name="small", bufs=1))

        stack = []

        def open_pool(name, bufs=1, space="SBUF"):
            cm = tc.tile_pool(name=name, bufs=bufs, space=space)
            pool = cm.__enter__()
            stack.append((name, cm))
            return pool

        def close_pool(name):
            n, cm = stack.pop()
            assert n == name, (n, name)
            cm.__exit__(None, None, None)

        # ---- const loads (persistent, small) ----
        tri_sb = cp.tile([P, P], f32)
        nc.sync.dma_start(out=tri_sb[:], in_=tri_d[:])
        tri16_sb = cp.tile([16, 16], f32)
        nc.sync.dma_start(out=tri16_sb[:], in_=tri16_d[:])
        ones_sb = cp.tile([P, P], f32)
        nc.sync.dma_start(out=ones_sb[:], in_=ones_d[:])
        ident_sb = cp.tile([P, P], f32)
        nc.sync.dma_start(out=ident_sb[:], in_=ident_d[:])
        liota_sb = cp.tile([P, TF], i32)
        nc.sync.dma_start(out=liota_sb[:], in_=liota_d[:])
        liotaf_sb = cp.tile([P, TF], f32)
        nc.sync.dma_start(out=liotaf_sb[:], in_=liotaf_d[:])
        maskI_sb = cp.tile([P, TF], i32)
        nc.sync.dma_start(out=maskI_sb[:], in_=mask_d[:])
        noise_sb = cp.tile([P, TF], f32)
        nc.sync.dma_start(out=noise_sb[:], in_=noise_d[:])
        lastm_sb = cp.tile([P, TF], f32)
        nc.sync.dma_start(out=lastm_sb[:], in_=lastm_d[:])

        pm_ = open_pool("pm")
        m_sb = pm_.tile([P, 8, D], f32, tag="m")
        ph = open_pool("ph")
        h_sb = ph.tile([P, T, D], f32, tag="h")
        nc.sync.dma_start(out=h_sb[:], in_=h_d.rearrange("(t p) d -> p t d", p=P))

        # prefill e_buf / pmask / Pp row 0
        pz = open_pool("pz")
        nc.sync.dma_start(out=e_buf[:], in_=efill_d[:])
        zrow = pz.tile([1, D], f32, tag="zrow")
        nc.gpsimd.memset(zrow[:], 0.0)
        nc.sync.dma_start(out=Pp[0:1, :], in_=zrow[:])
        close_pool("pz")

        # ---- M = Wq^T @ Wk : M[j, d] ; Wk streamed in two d-halves ----
        pwq = open_pool("pwq")
        wq_sb = pwq.tile([P, 8, D], f32, tag="wq")
        nc.sync.dma_start(out=wq_sb[:], in_=wq_d.rearrange("(tt p) j -> p tt j", p=P))
        ps_mm = open_pool("ps_mm", bufs=8, space="PSUM")
        pwkh = open_pool("pwkh", bufs=1)
        for half in range(2):
            wkh = pwkh.tile([P, 8, 512], f32, tag="wkh")
            nc.sync.dma_start(
                out=wkh[:],
                in_=wk_d[:, half * 512:(half + 1) * 512].rearrange(
                    "(tt p) j -> p tt j", p=P),
            )
            mts = [ps_mm.tile([P, 512], f32, tag="mps", name=f"mps{half}_{j}") for j in range(8)]
            for kt in range(8):
                for jc in range(8):
                    nc.tensor.matmul(
                        mts[jc][:],
                        lhsT=wq_sb[:, kt, jc * P:(jc + 1) * P],
                        rhs=wkh[:, kt, :],
                        start=(kt == 0), stop=(kt == 7),
                    )
            for jc in range(8):
                dst = m_sb[:, jc, half * 512:(half + 1) * 512]
                if jc % 2 == 0:
                    nc.vector.tensor_copy(dst, mts[jc][:])
                else:
                    nc.scalar.activation(dst, mts[jc][:], AF.Copy)
        close_pool("pwkh")
        close_pool("ps_mm")
        close_pool("pwq")

        # ---- HT = h^T : [128(j-chunk), 2048(l)] ----
        pht = open_pool("pht")
        ht_sb = pht.tile([P, 8, L], f32, tag="ht")
        ps_tr = open_pool("ps_tr", bufs=4, space="PSUM")
        for t in range(T):
            for jc in range(8):
                tp_ps = ps_tr.tile([P, P], f32, tag="tps")
                nc.tensor.transpose(
                    out=tp_ps[:], in_=h_sb[:, t, jc * P:(jc + 1) * P],
                    identity=ident_sb[:],
                )
                if (t + jc) % 2 == 0:
                    nc.vector.tensor_copy(ht_sb[:, jc, t * P:(t + 1) * P], tp_ps[:])
                else:
                    nc.scalar.activation(ht_sb[:, jc, t * P:(t + 1) * P], tp_ps[:], AF.Copy)
        close_pool("ps_tr")

        # ---- nrm2[l] = sum_d HT[d, l]^2 ----
        plin = open_pool("plin")
        psq = open_pool("psq", bufs=3)
        ps_n2 = open_pool("ps_n2", bufs=1, space="PSUM")
        n2_ps = ps_n2.tile([1, L], f32, tag="n2ps")
        for dc in range(8):
            for st in range(4):
                sq = psq.tile([P, 512], f32, tag="sq")
                nc.scalar.activation(sq[:], ht_sb[:, dc, st * 512:(st + 1) * 512], AF.Square)
                nc.tensor.matmul(
                    n2_ps[0:1, st * 512:(st + 1) * 512],
                    lhsT=ones_sb[:, 0:1],
                    rhs=sq[:],
                    start=(dc == 0), stop=(dc == 7),
                )
        n2lin = plin.tile([1, L], f32, tag="n2lin")
        nc.vector.tensor_copy(n2lin[:], n2_ps[:])
        close_pool("ps_n2")
        close_pool("psq")

        # ---- UT + praw ----
        wpr = open_pool("wpr", bufs=3)
        ps_pr = open_pool("ps_pr", bufs=1, space="PSUM")
        ps_ut = open_pool("ps_ut", bufs=2, space="PSUM")
        pr_ps = ps_pr.tile([1, L], f32, tag="prps")   # cols 0..2046 used
        strips = [(0, 512), (512, 512), (1024, 512), (1536, 511)]
        for dc in range(8):
            for (s0, sn) in strips:
                ut = ps_ut.tile([P, 512], f32, tag="utps")
                for jt in range(8):
                    nc.tensor.matmul(
                        ut[:, 0:sn],
                        lhsT=m_sb[:, jt, dc * P:(dc + 1) * P],
                        rhs=ht_sb[:, jt, s0:s0 + sn],
                        start=(jt == 0), stop=(jt == 7),
                    )
                pr = wpr.tile([P, 512], f32, tag="pr")
                nc.vector.tensor_tensor(
                    out=pr[:, 0:sn], in0=ut[:, 0:sn],
                    in1=ht_sb[:, dc, s0 + 1:s0 + sn + 1], op=A.mult,
                )
                nc.tensor.matmul(
                    pr_ps[0:1, s0:s0 + sn],
                    lhsT=ones_sb[:, 0:1],
                    rhs=pr[:, 0:sn],
                    start=(dc == 0), stop=(dc == 7),
                )
        close_pool("ps_ut")

        # stage[l] = praw[l-1] for l>=1, stage[0] = -1
        stage = plin.tile([1, L], f32, tag="stage")
        nc.scalar.activation(stage[0:1, 1:L], pr_ps[0:1, 0:L - 1], AF.Copy)
        stm1 = sp.tile([1, 1], f32)
        nc.gpsimd.memset(stm1[:], -1.0)
        nc.vector.tensor_copy(stage[0:1, 0:1], stm1[:])
        close_pool("ps_pr")
        close_pool("wpr")

        # ---- fold praw/nrm2 to F2 [128, 16] via DRAM staging ----
        nc.sync.dma_start(out=stg_dram[:], in_=stage[:])
        nc.sync.dma_start(out=n2x_dram[0:1, 1:L + 1], in_=n2lin[:])
        one1a = sp.tile([1, 1], f32)
        nc.gpsimd.memset(one1a[:], 1.0)
        nc.sync.dma_start(out=n2x_dram[0:1, 0:1], in_=one1a[:])
        stF2 = sp.tile([P, TF], f32)
        nc.sync.dma_start(out=stF2[:], in_=stg_dram.rearrange("o (p t) -> (o p) t", p=P))
        n2F2 = sp.tile([P, TF], f32)
        nc.sync.dma_start(out=n2F2[:], in_=n2x_dram[0:1, 1:L + 1].rearrange("o (p t) -> (o p) t", p=P))
        # n2back[p] = n2x[16p] = nrm2[16p - 1], with n2x[0] = 1
        n2back = sp.tile([P, 1], f32)
        nc.sync.dma_start(out=n2back[:], in_=n2x_dram[0:1, 0:L].rearrange("o (p t) -> (o p) t", p=P)[:, 0:1])
        close_pool("plin")
        close_pool("pht")

        # prod[l] = nrm2[l-1] * nrm2[l]; prod[0] = 1
        prod = sp.tile([P, TF], f32)
        nc.vector.tensor_tensor(prod[:, 1:TF], n2F2[:, 1:TF], n2F2[:, 0:TF - 1], A.mult)
        nc.vector.tensor_tensor(prod[:, 0:1], n2F2[:, 0:1], n2back[:], A.mult)
        one1 = sp.tile([1, 1], f32)
        nc.gpsimd.memset(one1[:], 1.0)
        nc.vector.tensor_copy(prod[0:1, 0:1], one1[:])

        # y = rsqrt(prod): reciprocal, ACT sqrt, 2 Newton steps
        rp = sp.tile([P, TF], f32)
        nc.vector.reciprocal(rp[:], prod[:])
        y = sp.tile([P, TF], f32)
        nc.scalar.activation(y[:], rp[:], AF.Sqrt)
        for _ in range(2):
            a_ = sp.tile([P, TF], f32, tag="nsq_a")
            nc.vector.tensor_tensor(a_[:], y[:], y[:], A.mult)
            nc.vector.tensor_tensor(a_[:], a_[:], prod[:], A.mult)
            nc.vector.tensor_scalar(a_[:], a_[:], -0.5, 1.5, A.mult, A.add)
            nc.vector.tensor_tensor(y[:], y[:], a_[:], A.mult)

        cosF2 = sp.tile([P, TF], f32)
        nc.vector.tensor_tensor(cosF2[:], stF2[:], y[:], A.mult)

        # ---- sampling: hard = (clip((1-cos)/2) + clip(u)) > 1 ----
        pclip = sp.tile([P, TF], f32)
        nc.vector.tensor_scalar(pclip[:], cosF2[:], -0.5, 0.5, A.mult, A.add)
        nc.vector.tensor_scalar(pclip[:], pclip[:], EPS, 1.0 - EPS, A.max, A.min)
        uclip = sp.tile([P, TF], f32)
        nc.vector.tensor_scalar(uclip[:], noise_sb[:], EPS, 1.0 - EPS, A.max, A.min)
        hard = sp.tile([P, TF], f32)
        nc.vector.tensor_tensor(hard[:], pclip[:], uclip[:], A.add)
        nc.vector.tensor_scalar(hard[:], hard[:], 1.0, None, A.is_gt)
        maskF = sp.tile([P, TF], f32)
        nc.vector.tensor_copy(maskF[:], maskI_sb[:])
        hardm = sp.tile([P, TF], f32)
        nc.vector.tensor_tensor(hardm[:], hard[:], maskF[:], A.mult)
        hb = sp.tile([P, TF], f32)
        nc.vector.tensor_scalar(hb[:], hardm[:], 0.5, None, A.is_gt)
        hbp = sp.tile([P, TF], f32)
        nc.vector.tensor_tensor(hbp[:], hb[:], lastm_sb[:], A.max)

        # ---- partial counts + allreduce ----
        ps_sc = open_pool("ps_sc", bufs=1, space="PSUM")
        mnz = sp.tile([P, TF], f32)
        nc.vector.tensor_scalar(mnz[:], maskF[:], 0.0, None, A.not_equal)
        red2 = sp.tile([P, 2], f32)
        nc.vector.tensor_reduce(red2[:, 0:1], hardm[:], mybir.AxisListType.X, A.add)
        nc.vector.tensor_reduce(red2[:, 1:2], mnz[:], mybir.AxisListType.X, A.add)
        sc_ps = ps_sc.tile([1, 2], f32, tag="scps")
        nc.tensor.matmul(sc_ps[:], lhsT=ones_sb[:, 0:1], rhs=red2[:], start=True, stop=True)
        sc_sb = sp.tile([1, 2], f32)
        nc.vector.tensor_copy(sc_sb[:], sc_ps[:])
        nc.sync.dma_start(out=cc_in[:], in_=sc_sb[:])
        import os
        if os.environ.get("K_NO_CC"):
            nc.sync.dma_start(out=cc_out[:], in_=cc_in[:])
        else:
            nc.gpsimd.collective_compute(
                "AllReduce", A.add,
                ins=[cc_in[:]], outs=[cc_out[:]],
                replica_groups=[list(range(NC))],
            )
        scg = sp.tile([1, 2], f32)
        nc.sync.dma_start(out=scg[:], in_=cc_out[:])

        # ---- inclusive cumsum of hbp over l ----
        colc = sp.tile([P, TF], f32)
        nc.vector.tensor_tensor_scan(colc[:], hbp[:], hbp[:], 0.0, A.add, A.bypass)
        rows = sp.tile([P, 1], f32)
        nc.vector.tensor_copy(rows[:], colc[:, TF - 1:TF])
        off_ps = ps_sc.tile([P, 1], f32, tag="offps")
        nc.tensor.matmul(off_ps[:], lhsT=tri_sb[:], rhs=rows[:], start=True, stop=True)
        # tri_sb is inclusive (k <= m); make exclusive by subtracting own row
        off_sb = sp.tile([P, 1], f32)
        nc.vector.tensor_tensor(off_sb[:], off_ps[:], rows[:], A.subtract)
        cum = sp.tile([P, TF], f32)
        nc.vector.tensor_scalar(cum[:], colc[:], off_sb[:, 0:1], None, A.add)

        # totals broadcast: nbT (forced count) and nbO (original count)
        dl = sp.tile([P, TF], f32)
        nc.vector.tensor_tensor(dl[:], hbp[:], hb[:], A.subtract)
        nc.vector.tensor_tensor(dl[:], dl[:], lastm_sb[:], A.mult)
        cum_hb = sp.tile([P, TF], f32)
        nc.vector.tensor_tensor(cum_hb[:], cum[:], dl[:], A.subtract)
        rd2 = sp.tile([P, 2], f32)
        nc.vector.tensor_copy(rd2[:, 0:1], rows[:])
        nc.vector.tensor_reduce(rd2[:, 1:2], dl[:], mybir.AxisListType.X, A.add)
        tot_ps = ps_sc.tile([P, 2], f32, tag="totps")
        nc.tensor.matmul(tot_ps[:], lhsT=ones_sb[:], rhs=rd2[:], start=True, stop=True)
        tot_sb = sp.tile([P, 2], f32)
        nc.vector.tensor_copy(tot_sb[:], tot_ps[:])
        nbO_sb = sp.tile([P, 1], f32)
        nc.vector.tensor_tensor(nbO_sb[:], tot_sb[:, 0:1], tot_sb[:, 1:2], A.subtract)
        close_pool("ps_sc")

        # bijective scatter offsets: boundary -> rank slot, non-boundary -> unused slot
        def bij_idx(flag, cumv, nb_ap, base_off, tag):
            # offB = (l - cum) + nbT + base_off ; offA = cum + base_off - 1
            offb = sp.tile([P, TF], f32, tag=tag + "_b", name=tag + "_b")
            nc.vector.tensor_tensor(offb[:], liotaf_sb[:], cumv[:], A.subtract)
            nc.vector.tensor_scalar(offb[:], offb[:], nb_ap, float(base_off), A.add, A.add)
            d_ = sp.tile([P, TF], f32, tag=tag + "_d", name=tag + "_d")
            nc.vector.tensor_tensor(d_[:], cumv[:], offb[:], A.subtract)
            nc.vector.tensor_scalar(d_[:], d_[:], float(base_off) - 1.0, None, A.add)
            nc.vector.tensor_tensor(d_[:], d_[:], flag[:], A.mult)
            nc.vector.tensor_tensor(offb[:], offb[:], d_[:], A.add)
            ti = sp.tile([P, TF], i32, tag=tag + "_i", name=tag + "_i")
            nc.vector.tensor_copy(ti[:], offb[:])
            return ti

        idx_e = bij_idx(hbp, cum, tot_sb[:, 0:1], 1.0, "idxe")
        idx_m = bij_idx(hb, cum_hb, nbO_sb[:, 0:1], 0.0, "idxm")
        pmv = sp.tile([P, TF], f32)
        nc.vector.tensor_tensor(pmv[:], maskF[:], hb[:], A.mult)
        pmvi = sp.tile([P, TF], i32)
        nc.vector.tensor_copy(pmvi[:], pmv[:])
        for t in range(TF):
            nc.gpsimd.indirect_dma_start(
                out=e_buf[:],
                out_offset=bass.IndirectOffsetOnAxis(ap=idx_e[:, t:t + 1], axis=0),
                in_=liota_sb[:, t:t + 1], in_offset=None,
            )
            nc.gpsimd.indirect_dma_start(
                out=pmask_d[:],
                out_offset=bass.IndirectOffsetOnAxis(ap=idx_m[:, t:t + 1], axis=0),
                in_=pmvi[:, t:t + 1], in_offset=None,
            )

        # ---- prefix sums of h into Pp (rows 1..2048) ----
        pcc = open_pool("pcc")
        sel_sb = pcc.tile([P, 16, 16], f32, tag="sel")
        nc.sync.dma_start(out=sel_sb[:], in_=sel_d[:])
        selrow_sb = pcc.tile([16, 16, P], f32, tag="selrow")
        nc.sync.dma_start(out=selrow_sb[:], in_=selrow_d[:])
        ps_bs = open_pool("ps_bs", bufs=1, space="PSUM")
        bs_ps = ps_bs.tile([16, D], f32, tag="bsps")
        for t in range(T):
            for st in range(2):
                nc.tensor.matmul(
                    bs_ps[:, st * 512:(st + 1) * 512],
                    lhsT=sel_sb[:, t, :],
                    rhs=h_sb[:, t, st * 512:(st + 1) * 512],
                    start=(t == 0), stop=(t == T - 1),
                )
        bs_sb = pcc.tile([16, D], f32, tag="bs_sb")
        nc.vector.tensor_copy(bs_sb[:], bs_ps[:])
        close_pool("ps_bs")
        ps_of = open_pool("ps_of", bufs=1, space="PSUM")
        of_ps = ps_of.tile([16, D], f32, tag="ofps")
        for st in range(2):
            nc.tensor.matmul(
                of_ps[:, st * 512:(st + 1) * 512],
                lhsT=tri16_sb[:],
                rhs=bs_sb[:, st * 512:(st + 1) * 512],
                start=True, stop=True,
            )
        of_sb = pcc.tile([16, D], f32, tag="of_sb")
        nc.vector.tensor_copy(of_sb[:], of_ps[:])
        close_pool("ps_of")
        ps_pf = open_pool("ps_pf", bufs=2, space="PSUM")
        wpf = open_pool("wpf", bufs=2)
        for t in range(T):
            pf = ps_pf.tile([P, D], f32, tag="pfps")
            for st in range(2):
                nc.tensor.matmul(
                    pf[:, st * 512:(st + 1) * 512],
                    lhsT=tri_sb[:],
                    rhs=h_sb[:, t, st * 512:(st + 1) * 512],
                    start=True, stop=False,
                )
                nc.tensor.matmul(
                    pf[:, st * 512:(st + 1) * 512],
                    lhsT=selrow_sb[:, t, :],
                    rhs=of_sb[:, st * 512:(st + 1) * 512],
                    start=False, stop=True,
                )
            pfs = wpf.tile([P, D], f32, tag="pfsb")
            if t % 2 == 0:
                nc.vector.tensor_copy(pfs[:], pf[:])
            else:
                nc.scalar.activation(pfs[:], pf[:], AF.Copy)
            nc.sync.dma_start(out=Pp[1 + t * P:1 + (t + 1) * P, :], in_=pfs[:])
        close_pool("wpf")
        close_pool("ps_pf")
        close_pool("pcc")

        # ---- read back e (+ shifted), counts, gather offsets ----
        eF2 = sp.tile([P, TF], i32)
        nc.sync.dma_start(out=eF2[:], in_=e_buf[1:L + 1, :].rearrange("(p t) o -> p (t o)", p=P))
        eSH = sp.tile([P, TF], i32)
        nc.sync.dma_start(out=eSH[:], in_=e_buf[0:L, :].rearrange("(p t) o -> p (t o)", p=P))
        ef = sp.tile([P, TF], f32)
        nc.vector.tensor_copy(ef[:], eF2[:])
        efs = sp.tile([P, TF], f32)
        nc.vector.tensor_copy(efs[:], eSH[:])

        # 17-column gather offsets: col 0 = e[j-1]+1 (sentinel -> 0), cols 1..16 = e[j]+1
        gb17 = sp.tile([P, TF + 1], f32)
        nc.vector.tensor_copy(gb17[:, 0:1], efs[:, 0:1])
        nc.vector.tensor_copy(gb17[:, 1:TF + 1], ef[:])
        nc.vector.tensor_scalar(gb17[:], gb17[:], 1.0, None, A.add)
        gbi = sp.tile([P, TF + 1], i32)
        nc.vector.tensor_copy(gbi[:], gb17[:])

        cnt = sp.tile([P, TF], f32)
        nc.vector.tensor_tensor(cnt[:], ef[:], efs[:], A.subtract)
        rcnt = sp.tile([P, TF], f32)
        nc.vector.tensor_scalar(rcnt[:], cnt[:], 1e-9, None, A.add)
        nc.vector.reciprocal(rcnt[:], rcnt[:])
        valid = sp.tile([P, TF], f32)
        nc.vector.tensor_scalar(valid[:], liotaf_sb[:], tot_sb[:, 0:1], None, A.is_lt)
        nc.vector.tensor_tensor(rcnt[:], rcnt[:], valid[:], A.mult)

        # ---- loss (Stirling lgamma) from allreduced scalars ----
        nbv = scg[0:1, 0:1]
        tpv = scg[0:1, 1:2]
        x3 = sp.tile([1, 3], f32)
        nc.vector.tensor_scalar(x3[0:1, 0:1], tpv, 1.0, None, A.add)
        nc.vector.tensor_scalar(x3[0:1, 1:2], nbv, 1.0, None, A.add)
        nc.vector.tensor_tensor(x3[0:1, 2:3], tpv, nbv, A.subtract)
        nc.vector.tensor_scalar(x3[0:1, 2:3], x3[0:1, 2:3], 1.0, None, A.add)
        ln3 = sp.tile([1, 3], f32)
        nc.scalar.activation(ln3[:], x3[:], AF.Ln)
        lg = sp.tile([1, 3], f32)
        nc.vector.tensor_scalar(lg[:], x3[:], -0.5, None, A.add)
        nc.vector.tensor_tensor(lg[:], lg[:], ln3[:], A.mult)
        nc.vector.tensor_tensor(lg[:], lg[:], x3[:], A.subtract)
        nc.vector.tensor_scalar(lg[:], lg[:], HALF_LN_2PI, None, A.add)
        t12 = sp.tile([1, 3], f32)
        nc.vector.tensor_scalar(t12[:], x3[:], 12.0, None, A.mult)
        nc.vector.reciprocal(t12[:], t12[:])
        nc.vector.tensor_tensor(lg[:], lg[:], t12[:], A.add)
        x3c = sp.tile([1, 3], f32)
        nc.vector.tensor_tensor(x3c[:], x3[:], x3[:], A.mult)
        nc.vector.tensor_tensor(x3c[:], x3c[:], x3[:], A.mult)
        nc.vector.tensor_scalar(x3c[:], x3c[:], 360.0, None, A.mult)
        nc.vector.reciprocal(x3c[:], x3c[:])
        nc.vector.tensor_tensor(lg[:], lg[:], x3c[:], A.subtract)

        lpm = sp.tile([1, 1], f32)
        nc.vector.tensor_tensor(lpm[:], lg[0:1, 0:1], lg[0:1, 1:2], A.subtract)
        nc.vector.tensor_tensor(lpm[:], lpm[:], lg[0:1, 2:3], A.subtract)
        w1 = sp.tile([1, 1], f32)
        nc.vector.tensor_scalar(w1[:], nbv, LOG02, None, A.mult)
        nc.vector.tensor_tensor(lpm[:], lpm[:], w1[:], A.add)
        w2 = sp.tile([1, 1], f32)
        nc.vector.tensor_tensor(w2[:], tpv, nbv, A.subtract)
        nc.vector.tensor_scalar(w2[:], w2[:], LOG08, None, A.mult)
        nc.vector.tensor_tensor(lpm[:], lpm[:], w2[:], A.add)
        rtp = sp.tile([1, 1], f32)
        nc.vector.reciprocal(rtp[:], tpv)
        nc.vector.tensor_tensor(lpm[:], lpm[:], rtp[:], A.mult)
        nc.vector.tensor_scalar(lpm[:], lpm[:], -1.0, None, A.mult)

        nc.sync.dma_start(out=loss_d[:], in_=lpm[:])
        nc.sync.dma_start(out=nb_d[:], in_=nbv)
        nc.sync.dma_start(out=tp_d[:], in_=tpv)
        close_pool("ph")
        close_pool("pm")

        # ---- gather prefix rows: G[p, 1+t, :] = Pp[e[16p+t]+1], G[p, 0] = Pp[e[j-1]+1] ----
        pg = open_pool("pg")
        G = pg.tile([P, TF + 1, D], f32, tag="G")
        for t in range(TF + 1):
            nc.gpsimd.indirect_dma_start(
                out=G[:, t, :], out_offset=None,
                in_=Pp[:], in_offset=bass.IndirectOffsetOnAxis(ap=gbi[:, t:t + 1], axis=0),
            )

        # ---- pooled[:, t] = (G[t+1] - G[t]) * rcnt[t], in place (descending t) ----
        for t in range(TF - 1, -1, -1):
            nc.vector.tensor_tensor(G[:, t + 1, :], G[:, t + 1, :], G[:, t, :], A.subtract)
        for t in range(TF):
            nc.vector.tensor_scalar(G[:, t + 1, :], G[:, t + 1, :], rcnt[:, t:t + 1], None, A.mult)
        nc.sync.dma_start(out=pooled_d.rearrange("(p t) d -> p t d", p=P), in_=G[:, 1:TF + 1, :])
        close_pool("pg")

    nc.compile()
    return nc


def _get_built():
    global _BUILT
    if _BUILT is None:
        _BUILT = _build()
    return _BUILT


def kernel(hidden, attention_mask, noise, Wq, Wk):
    from concourse.bass_utils import run_bass_kernel_spmd

    nc = _get_built()
    B = hidden.shape[0]
    assert B == NC
    consts = _consts()
    hidden = np.ascontiguousarray(hidden, np.float32)
    Wq = np.ascontiguousarray(Wq, np.float32)
    Wk = np.ascontiguousarray(Wk, np.float32)
    in_maps = []
    for c in range(NC):
        m = dict(consts)
        m["h"] = hidden[c]
        m["Wq"] = Wq
        m["Wk"] = Wk
        m["mask_f2"] = np.ascontiguousarray(attention_mask[c].reshape(P, TF), np.int32)
        m["noise_f2"] = np.ascontiguousarray(noise[c].reshape(P, TF), np.float32)
        in_maps.append(m)
    res = run_bass_kernel_spmd(nc, in_maps, list(range(NC))).results

    pooled = np.stack([res[c]["pooled"] for c in range(NC)])
    pmask = np.stack([res[c]["pmask"].reshape(L) for c in range(NC)]).astype(np.int32)
    loss = np.float32(res[0]["loss"].reshape(())[()])
    nb = np.float32(res[0]["nb"].reshape(())[()])
    tp = np.float32(res[0]["tp"].reshape(())[()])
    return pooled, pmask, loss, nb, tp


# revision 20
# speedup vs baseline: 1.5445x; 1.5445x over previous
"""Trainium2 Bass kernel for nn_BoundaryPredictor2 (segment_reduce).

Data-parallel over batch B=8 across 8 NeuronCores; only the scalar
num_boundaries / total_positions are all-reduced.

Per-core algorithm (batch row h [L=2048, D=1024], fp32 throughout):
  M = Wq^T @ Wk                       (PE, [D, D])
  HT = h^T                            (PE transposes)
  UT[d, l] = sum_j M[j, d] HT[j, l]   (PE; = (h @ M)^T)
  praw[l]  = sum_d UT[d, l] HT[d, l+1]   (DVE mul fused w/ PSUM evac + PE ones-reduce)
  nrm2[l]  = sum_d HT[d, l]^2            (ACT square + PE ones-reduce)
  cos[l]   = praw[l-1] * rsqrt(nrm2[l-1] nrm2[l]);  cos[0] = -1
  hard     = (clip((1-cos)/2, eps, 1-eps) + clip(noise, eps, 1-eps)) > 1
             -- algebraically identical to sigmoid(logit(p)+logit(u)) > 1/2
  seg ids via cumsum of hard (trailing boundary forced at L-1), boundary
  positions e[j] scattered by rank (indirect DMA), block prefix sums of h
  (PE triangular matmuls + cross-block offsets), pooled[j] =
  (P[e_j] - P[e_{j-1}]) / (e_j - e_{j-1}) via indirect row gather + shifted
  diffs.  pooled_mask scattered likewise.  Scalars all-reduced; loss from a
  Stirling lgamma on-device.
"""

import numpy as np

L = 2048
D = 1024
P = 128
T = L // P           # 16 l-blocks of 128 (natural fold)
TF = L // P          # 16 cols in the (p-major) F2 fold: l = 16*p + t
NC = 8
EPS = 1.1920929e-07
LOG02 = float(np.log(np.float32(0.2)))
LOG08 = float(np.log1p(np.float32(-0.2)))
HALF_LN_2PI = float(np.float32(0.5 * np.log(2.0 * np.pi)))

_BUILT = None


def _consts():
    tri_incl = (np.arange(P)[:, None] <= np.arange(P)[None, :]).astype(np.float32)
    tri16s = (np.arange(16)[:, None] < np.arange(16)[None, :]).astype(np.float32)
    sel = np.zeros((P, 16, 16), np.float32)
    for t in range(16):
        sel[:, t, t] = 1.0
    selrow = np.zeros((16, 16, P), np.float32)
    for t in range(16):
        selrow[t, t, :] = 1.0
    ones = np.ones((P, P), np.float32)
    ident = np.eye(P, dtype=np.float32)
    liota = (np.arange(P)[:, None] * TF + np.arange(TF)[None, :]).astype(np.int32)
    efill = np.full((L + 1, 1), L - 1, np.int32)
    efill[0, 0] = -1
    liota_f = (np.arange(P)[:, None] * TF + np.arange(TF)[None, :]).astype(np.float32)
    lastm = np.zeros((P, TF), np.float32)
    lastm[P - 1, TF - 1] = 1.0
    return dict(tri_incl=tri_incl, tri16s=tri16s, sel=sel, selrow=selrow,
                ones=ones, ident=ident, liota=liota, liota_f=liota_f, efill=efill, lastm=lastm)


def _build():
    import concourse.bacc as bacc
    import concourse.mybir as mybir
    import concourse.tile as tile
    import concourse.bass as bass

    f32 = mybir.dt.float32
    f32r = mybir.dt.float32r
    i32 = mybir.dt.int32
    A = mybir.AluOpType
    AF = mybir.ActivationFunctionType

    nc = bacc.Bacc("TRN2", target_bir_lowering=False, debug=False, num_devices=NC)

    # ---- dram parameters ----
    h_d = nc.declare_dram_parameter("h", [L, D], f32, isOutput=False)
    wq_d = nc.declare_dram_parameter("Wq", [D, D], f32r, isOutput=False)
    wk_d = nc.declare_dram_parameter("Wk", [D, D], f32r, isOutput=False)
    mask_d = nc.declare_dram_parameter("mask_f2", [P, TF], i32, isOutput=False)
    noise_d = nc.declare_dram_parameter("noise_f2", [P, TF], f32, isOutput=False)
    tri_d = nc.declare_dram_parameter("tri_incl", [P, P], f32, isOutput=False)
    tri16_d = nc.declare_dram_parameter("tri16s", [16, 16], f32, isOutput=False)
    sel_d = nc.declare_dram_parameter("sel", [P, 16, 16], f32, isOutput=False)
    selrow_d = nc.declare_dram_parameter("selrow", [16, 16, P], f32, isOutput=False)
    ones_d = nc.declare_dram_parameter("ones", [P, P], f32, isOutput=False)
    ident_d = nc.declare_dram_parameter("ident", [P, P], f32, isOutput=False)
    liota_d = nc.declare_dram_parameter("liota", [P, TF], i32, isOutput=False)
    liotaf_d = nc.declare_dram_parameter("liota_f", [P, TF], f32, isOutput=False)
    efill_d = nc.declare_dram_parameter("efill", [L + 1, 1], i32, isOutput=False)
    lastm_d = nc.declare_dram_parameter("lastm", [P, TF], f32, isOutput=False)

    pooled_d = nc.declare_dram_parameter("pooled", [L, D], f32, isOutput=True)
    pmask_d = nc.declare_dram_parameter("pmask", [L, 1], i32, isOutput=True)
    loss_d = nc.declare_dram_parameter("loss", [1, 1], f32, isOutput=True)
    nb_d = nc.declare_dram_parameter("nb", [1, 1], f32, isOutput=True)
    tp_d = nc.declare_dram_parameter("tp", [1, 1], f32, isOutput=True)

    # ---- internal dram ----
    Pp = nc.dram_tensor("Pp", [L + 1, D], f32)           # prefix, row 0 = 0
    e_buf = nc.dram_tensor("e_buf", [L + 1, 1], i32)   # row 0 = -1 sentinel
    stg_dram = nc.dram_tensor("stg_dram", [1, L], f32)
    n2x_dram = nc.dram_tensor("n2x_dram", [1, L + 1], f32)
    cc_in = nc.dram_tensor("cc_in", [1, 2], f32)
    cc_out = nc.dram_tensor("cc_out", [1, 2], f32, addr_space="Shared")

    from contextlib import ExitStack

    with tile.TileContext(nc) as tc, ExitStack() as ctx:
        cp = ctx.enter_context(tc.tile_pool(name="consts", bufs=1))
        sp = ctx.enter_context(tc.tile_pool(name="small", bufs=1))

        stack = []

        def open_pool(name, bufs=1, space="SBUF"):
            cm = tc.tile_pool(name=name, bufs=bufs, space=space)
            pool = cm.__enter__()
            stack.append((name, cm))
            return pool

        def close_pool(name):
            n, cm = stack.pop()
            assert n == name, (n, name)
            cm.__exit__(None, None, None)

        # ---- const loads (persistent, small) ----
        tri_sb = cp.tile([P, P], f32)
        nc.sync.dma_start(out=tri_sb[:], in_=tri_d[:])
        tri16_sb = cp.tile([16, 16], f32)
        nc.sync.dma_start(out=tri16_sb[:], in_=tri16_d[:])
        ones_sb = cp.tile([P, P], f32)
        nc.sync.dma_start(out=ones_sb[:], in_=ones_d[:])
        ident_sb = cp.tile([P, P], f32)
        nc.sync.dma_start(out=ident_sb[:], in_=ident_d[:])
        liota_sb = cp.tile([P, TF], i32)
        nc.sync.dma_start(out=liota_sb[:], in_=liota_d[:])
        liotaf_sb = cp.tile([P, TF], f32)
        nc.sync.dma_start(out=liotaf_sb[:], in_=liotaf_d[:])
        maskI_sb = cp.tile([P, TF], i32)
        nc.sync.dma_start(out=maskI_sb[:], in_=mask_d[:])
        noise_sb = cp.tile([P, TF], f32)
        nc.sync.dma_start(out=noise_sb[:], in_=noise_d[:])
        lastm_sb = cp.tile([P, TF], f32)
        nc.sync.dma_start(out=lastm_sb[:], in_=lastm_d[:])

        pm_ = open_pool("pm")
        m_sb = pm_.tile([P, 8, D], f32r, tag="m")
        ph = open_pool("ph")
        h_sb = ph.tile([P, T, D], f32, tag="h")
        nc.sync.dma_start(out=h_sb[:], in_=h_d.rearrange("(t p) d -> p t d", p=P))

        # prefill e_buf / pmask / Pp row 0
        pz = open_pool("pz")
        nc.sync.dma_start(out=e_buf[:], in_=efill_d[:])
        zrow = pz.tile([1, D], f32, tag="zrow")
        nc.gpsimd.memset(zrow[:], 0.0)
        nc.sync.dma_start(out=Pp[0:1, :], in_=zrow[:])
        close_pool("pz")

        # ---- M = Wq^T @ Wk : M[j, d] ; Wk streamed in two d-halves ----
        pwq = open_pool("pwq")
        wq_sb = pwq.tile([P, 8, D], f32r, tag="wq")
        nc.sync.dma_start(out=wq_sb[:], in_=wq_d.rearrange("(tt p) j -> p tt j", p=P))
        ps_mm = open_pool("ps_mm", bufs=8, space="PSUM")
        pwkh = open_pool("pwkh", bufs=1)
        for half in range(2):
            wkh = pwkh.tile([P, 8, 512], f32r, tag="wkh")
            nc.sync.dma_start(
                out=wkh[:],
                in_=wk_d[:, half * 512:(half + 1) * 512].rearrange(
                    "(tt p) j -> p tt j", p=P),
            )
            mts = [ps_mm.tile([P, 512], f32, tag="mps", name=f"mps{half}_{j}") for j in range(8)]
            for kt in range(8):
                for jc in range(8):
                    nc.tensor.matmul(
                        mts[jc][:],
                        lhsT=wq_sb[:, kt, jc * P:(jc + 1) * P],
                        rhs=wkh[:, kt, :],
                        start=(kt == 0), stop=(kt == 7),
                    )
            for jc in range(8):
                dst = m_sb[:, jc, half * 512:(half + 1) * 512]
                if jc % 2 == 0:
                    nc.vector.tensor_copy(dst, mts[jc][:])
                else:
                    nc.scalar.activation(dst, mts[jc][:], AF.Copy)
        close_pool("pwkh")
        close_pool("ps_mm")
        close_pool("pwq")

        # ---- HT = h^T : [128(j-chunk), 2048(l)] ----
        pht = open_pool("pht")
        ht_sb = pht.tile([P, 8, L], f32r, tag="ht")
        ps_tr = open_pool("ps_tr", bufs=4, space="PSUM")
        for t in range(T):
            for jc in range(8):
                tp_ps = ps_tr.tile([P, P], f32, tag="tps")
                nc.tensor.transpose(
                    out=tp_ps[:], in_=h_sb[:, t, jc * P:(jc + 1) * P],
                    identity=ident_sb[:],
                )
                if (t + jc) % 2 == 0:
                    nc.vector.tensor_copy(ht_sb[:, jc, t * P:(t + 1) * P], tp_ps[:])
                else:
                    nc.scalar.activation(ht_sb[:, jc, t * P:(t + 1) * P], tp_ps[:], AF.Copy)
        close_pool("ps_tr")

        # ---- nrm2[l] = sum_d HT[d, l]^2 ----
        plin = open_pool("plin")
        psq = open_pool("psq", bufs=3)
        ps_n2 = open_pool("ps_n2", bufs=1, space="PSUM")
        n2_ps = ps_n2.tile([1, L], f32, tag="n2ps")
        for dc in range(8):
            for st in range(4):
                sq = psq.tile([P, 512], f32, tag="sq")
                nc.scalar.activation(sq[:], ht_sb[:, dc, st * 512:(st + 1) * 512].bitcast(f32), AF.Square)
                nc.tensor.matmul(
                    n2_ps[0:1, st * 512:(st + 1) * 512],
                    lhsT=ones_sb[:, 0:1],
                    rhs=sq[:],
                    start=(dc == 0), stop=(dc == 7),
                )
        n2lin = plin.tile([1, L], f32, tag="n2lin")
        nc.vector.tensor_copy(n2lin[:], n2_ps[:])
        close_pool("ps_n2")
        close_pool("psq")

        # ---- UT + praw ----
        wpr = open_pool("wpr", bufs=3)
        ps_pr = open_pool("ps_pr", bufs=1, space="PSUM")
        ps_ut = open_pool("ps_ut", bufs=2, space="PSUM")
        pr_ps = ps_pr.tile([1, L], f32, tag="prps")   # cols 0..2046 used
        strips = [(0, 512), (512, 512), (1024, 512), (1536, 512)]
        for dc in range(8):
            for (s0, sn) in strips:
                sn2 = sn if s0 + sn < L else sn - 1   # PR shift reads col l'+1
                ut = ps_ut.tile([P, 512], f32, tag="utps")
                for jt in range(8):
                    nc.tensor.matmul(
                        ut[:, 0:sn],
                        lhsT=m_sb[:, jt, dc * P:(dc + 1) * P],
                        rhs=ht_sb[:, jt, s0:s0 + sn],
                        start=(jt == 0), stop=(jt == 7),
                    )
                pr = wpr.tile([P, 512], f32, tag="pr")
                nc.vector.tensor_tensor(
                    out=pr[:, 0:sn2], in0=ut[:, 0:sn2],
                    in1=ht_sb[:, dc, s0 + 1:s0 + sn2 + 1].bitcast(f32), op=A.mult,
                )
                nc.tensor.matmul(
                    pr_ps[0:1, s0:s0 + sn2],
                    lhsT=ones_sb[:, 0:1],
                    rhs=pr[:, 0:sn2],
                    start=(dc == 0), stop=(dc == 7),
                )
        close_pool("ps_ut")

        # stage[l] = praw[l-1] for l>=1, stage[0] = -1
        stage = plin.tile([1, L], f32, tag="stage")
        nc.scalar.activation(stage[0:1, 1:L], pr_ps[0:1, 0:L - 1], AF.Copy)
        stm1 = sp.tile([1, 1], f32)
        nc.gpsimd.memset(stm1[:], -1.0)
        nc.vector.tensor_copy(stage[0:1, 0:1], stm1[:])
        close_pool("ps_pr")
        close_pool("wpr")

        # ---- fold praw/nrm2 to F2 [128, 16] via DRAM staging ----
        nc.sync.dma_start(out=stg_dram[:], in_=stage[:])
        nc.sync.dma_start(out=n2x_dram[0:1, 1:L + 1], in_=n2lin[:])
        one1a = sp.tile([1, 1], f32)
        nc.gpsimd.memset(one1a[:], 1.0)
        nc.sync.dma_start(out=n2x_dram[0:1, 0:1], in_=one1a[:])
        stF2 = sp.tile([P, TF], f32)
        nc.sync.dma_start(out=stF2[:], in_=stg_dram.rearrange("o (p t) -> (o p) t", p=P))
        n2F2 = sp.tile([P, TF], f32)
        nc.sync.dma_start(out=n2F2[:], in_=n2x_dram[0:1, 1:L + 1].rearrange("o (p t) -> (o p) t", p=P))
        # n2back[p] = n2x[16p] = nrm2[16p - 1], with n2x[0] = 1
        n2back = sp.tile([P, 1], f32)
        nc.sync.dma_start(out=n2back[:], in_=n2x_dram[0:1, 0:L].rearrange("o (p t) -> (o p) t", p=P)[:, 0:1])
        close_pool("plin")
        close_pool("pht")

        # prod[l] = nrm2[l-1] * nrm2[l]; prod[0] = 1
        prod = sp.tile([P, TF], f32)
        nc.vector.tensor_tensor(prod[:, 1:TF], n2F2[:, 1:TF], n2F2[:, 0:TF - 1], A.mult)
        nc.vector.tensor_tensor(prod[:, 0:1], n2F2[:, 0:1], n2back[:], A.mult)
        one1 = sp.tile([1, 1], f32)
        nc.gpsimd.memset(one1[:], 1.0)
        nc.vector.tensor_copy(prod[0:1, 0:1], one1[:])

        # y = rsqrt(prod): reciprocal, ACT sqrt, 2 Newton steps
        rp = sp.tile([P, TF], f32)
        nc.vector.reciprocal(rp[:], prod[:])
        y = sp.tile([P, TF], f32)
        nc.scalar.activation(y[:], rp[:], AF.Sqrt)
        for _ in range(2):
            a_ = sp.tile([P, TF], f32, tag="nsq_a")
            nc.vector.tensor_tensor(a_[:], y[:], y[:], A.mult)
            nc.vector.tensor_tensor(a_[:], a_[:], prod[:], A.mult)
            nc.vector.tensor_scalar(a_[:], a_[:], -0.5, 1.5, A.mult, A.add)
            nc.vector.tensor_tensor(y[:], y[:], a_[:], A.mult)

        cosF2 = sp.tile([P, TF], f32)
        nc.vector.tensor_tensor(cosF2[:], stF2[:], y[:], A.mult)

        # ---- sampling: hard = (clip((1-cos)/2) + clip(u)) > 1 ----
        pclip = sp.tile([P, TF], f32)
        nc.vector.tensor_scalar(pclip[:], cosF2[:], -0.5, 0.5, A.mult, A.add)
        nc.vector.tensor_scalar(pclip[:], pclip[:], EPS, 1.0 - EPS, A.max, A.min)
        uclip = sp.tile([P, TF], f32)
        nc.vector.tensor_scalar(uclip[:], noise_sb[:], EPS, 1.0 - EPS, A.max, A.min)
        hard = sp.tile([P, TF], f32)
        nc.vector.tensor_tensor(hard[:], pclip[:], uclip[:], A.add)
        nc.vector.tensor_scalar(hard[:], hard[:], 1.0, None, A.is_gt)
        maskF = sp.tile([P, TF], f32)
        nc.vector.tensor_copy(maskF[:], maskI_sb[:])
        hardm = sp.tile([P, TF], f32)
        nc.vector.tensor_tensor(hardm[:], hard[:], maskF[:], A.mult)
        hb = sp.tile([P, TF], f32)
        nc.vector.tensor_scalar(hb[:], hardm[:], 0.5, None, A.is_gt)
        hbp = sp.tile([P, TF], f32)
        nc.vector.tensor_tensor(hbp[:], hb[:], lastm_sb[:], A.max)

        # ---- partial counts + allreduce ----
        ps_sc = open_pool("ps_sc", bufs=1, space="PSUM")
        mnz = sp.tile([P, TF], f32)
        nc.vector.tensor_scalar(mnz[:], maskF[:], 0.0, None, A.not_equal)
        red2 = sp.tile([P, 2], f32)
        nc.vector.tensor_reduce(red2[:, 0:1], hardm[:], mybir.AxisListType.X, A.add)
        nc.vector.tensor_reduce(red2[:, 1:2], mnz[:], mybir.AxisListType.X, A.add)
        sc_ps = ps_sc.tile([1, 2], f32, tag="scps")
        nc.tensor.matmul(sc_ps[:], lhsT=ones_sb[:, 0:1], rhs=red2[:], start=True, stop=True)
        sc_sb = sp.tile([1, 2], f32)
        nc.vector.tensor_copy(sc_sb[:], sc_ps[:])
        nc.sync.dma_start(out=cc_in[:], in_=sc_sb[:])
        import os
        if os.environ.get("K_NO_CC"):
            nc.sync.dma_start(out=cc_out[:], in_=cc_in[:])
        else:
            nc.gpsimd.collective_compute(
                "AllReduce", A.add,
                ins=[cc_in[:]], outs=[cc_out[:]],
                replica_groups=[list(range(NC))],
            )
        scg = sp.tile([1, 2], f32)
        nc.sync.dma_start(out=scg[:], in_=cc_out[:])

        # ---- inclusive cumsum of hbp over l ----
        colc = sp.tile([P, TF], f32)
        nc.vector.tensor_tensor_scan(colc[:], hbp[:], hbp[:], 0.0, A.add, A.bypass)
        rows = sp.tile([P, 1], f32)
        nc.vector.tensor_copy(rows[:], colc[:, TF - 1:TF])
        off_ps = ps_sc.tile([P, 1], f32, tag="offps")
        nc.tensor.matmul(off_ps[:], lhsT=tri_sb[:], rhs=rows[:], start=True, stop=True)
        # tri_sb is inclusive (k <= m); make exclusive by subtracting own row
        off_sb = sp.tile([P, 1], f32)
        nc.vector.tensor_tensor(off_sb[:], off_ps[:], rows[:], A.subtract)
        cum = sp.tile([P, TF], f32)
        nc.vector.tensor_scalar(cum[:], colc[:], off_sb[:, 0:1], None, A.add)

        # totals broadcast: nbT (forced count) and nbO (original count)
        dl = sp.tile([P, TF], f32)
        nc.vector.tensor_tensor(dl[:], hbp[:], hb[:], A.subtract)
        nc.vector.tensor_tensor(dl[:], dl[:], lastm_sb[:], A.mult)
        cum_hb = sp.tile([P, TF], f32)
        nc.vector.tensor_tensor(cum_hb[:], cum[:], dl[:], A.subtract)
        rd2 = sp.tile([P, 2], f32)
        nc.vector.tensor_copy(rd2[:, 0:1], rows[:])
        nc.vector.tensor_reduce(rd2[:, 1:2], dl[:], mybir.AxisListType.X, A.add)
        tot_ps = ps_sc.tile([P, 2], f32, tag="totps")
        nc.tensor.matmul(tot_ps[:], lhsT=ones_sb[:], rhs=rd2[:], start=True, stop=True)
        tot_sb = sp.tile([P, 2], f32)
        nc.vector.tensor_copy(tot_sb[:], tot_ps[:])
        nbO_sb = sp.tile([P, 1], f32)
        nc.vector.tensor_tensor(nbO_sb[:], tot_sb[:, 0:1], tot_sb[:, 1:2], A.subtract)
        close_pool("ps_sc")

        # bijective scatter offsets: boundary -> rank slot, non-boundary -> unused slot
        def bij_idx(flag, cumv, nb_ap, base_off, tag):
            # offB = (l - cum) + nbT + base_off ; offA = cum + base_off - 1
            offb = sp.tile([P, TF], f32, tag=tag + "_b", name=tag + "_b")
            nc.vector.tensor_tensor(offb[:], liotaf_sb[:], cumv[:], A.subtract)
            nc.vector.tensor_scalar(offb[:], offb[:], nb_ap, float(base_off), A.add, A.add)
            d_ = sp.tile([P, TF], f32, tag=tag + "_d", name=tag + "_d")
            nc.vector.tensor_tensor(d_[:], cumv[:], offb[:], A.subtract)
            nc.vector.tensor_scalar(d_[:], d_[:], float(base_off) - 1.0, None, A.add)
            nc.vector.tensor_tensor(d_[:], d_[:], flag[:], A.mult)
            nc.vector.tensor_tensor(offb[:], offb[:], d_[:], A.add)
            ti = sp.tile([P, TF], i32, tag=tag + "_i", name=tag + "_i")
            nc.vector.tensor_copy(ti[:], offb[:])
            return ti

        idx_e = bij_idx(hbp, cum, tot_sb[:, 0:1], 1.0, "idxe")
        idx_m = bij_idx(hb, cum_hb, nbO_sb[:, 0:1], 0.0, "idxm")
        pmv = sp.tile([P, TF], f32)
        nc.vector.tensor_tensor(pmv[:], maskF[:], hb[:], A.mult)
        pmvi = sp.tile([P, TF], i32)
        nc.vector.tensor_copy(pmvi[:], pmv[:])
        for t in range(TF):
            nc.gpsimd.indirect_dma_start(
                out=e_buf[:],
                out_offset=bass.IndirectOffsetOnAxis(ap=idx_e[:, t:t + 1], axis=0),
                in_=liota_sb[:, t:t + 1], in_offset=None,
            )
            nc.gpsimd.indirect_dma_start(
                out=pmask_d[:],
                out_offset=bass.IndirectOffsetOnAxis(ap=idx_m[:, t:t + 1], axis=0),
                in_=pmvi[:, t:t + 1], in_offset=None,
            )

        # ---- prefix sums of h into Pp (rows 1..2048) ----
        pcc = open_pool("pcc")
        sel_sb = pcc.tile([P, 16, 16], f32, tag="sel")
        nc.sync.dma_start(out=sel_sb[:], in_=sel_d[:])
        selrow_sb = pcc.tile([16, 16, P], f32, tag="selrow")
        nc.sync.dma_start(out=selrow_sb[:], in_=selrow_d[:])
        ps_bs = open_pool("ps_bs", bufs=1, space="PSUM")
        bs_ps = ps_bs.tile([16, D], f32, tag="bsps")
        for t in range(T):
            for st in range(2):
                nc.tensor.matmul(
                    bs_ps[:, st * 512:(st + 1) * 512],
                    lhsT=sel_sb[:, t, :],
                    rhs=h_sb[:, t, st * 512:(st + 1) * 512],
                    start=(t == 0), stop=(t == T - 1),
                )
        bs_sb = pcc.tile([16, D], f32, tag="bs_sb")
        nc.vector.tensor_copy(bs_sb[:], bs_ps[:])
        close_pool("ps_bs")
        ps_of = open_pool("ps_of", bufs=1, space="PSUM")
        of_ps = ps_of.tile([16, D], f32, tag="ofps")
        for st in range(2):
            nc.tensor.matmul(
                of_ps[:, st * 512:(st + 1) * 512],
                lhsT=tri16_sb[:],
                rhs=bs_sb[:, st * 512:(st + 1) * 512],
                start=True, stop=True,
            )
        of_sb = pcc.tile([16, D], f32, tag="of_sb")
        nc.vector.tensor_copy(of_sb[:], of_ps[:])
        close_pool("ps_of")
        ps_pf = open_pool("ps_pf", bufs=2, space="PSUM")
        wpf = open_pool("wpf", bufs=2)
        for t in range(T):
            pf = ps_pf.tile([P, D], f32, tag="pfps")
            for st in range(2):
                nc.tensor.matmul(
                    pf[:, st * 512:(st + 1) * 512],
                    lhsT=tri_sb[:],
                    rhs=h_sb[:, t, st * 512:(st + 1) * 512],
                    start=True, stop=False,
                )
                nc.tensor.matmul(
                    pf[:, st * 512:(st + 1) * 512],
                    lhsT=selrow_sb[:, t, :],
                    rhs=of_sb[:, st * 512:(st + 1) * 512],
                    start=False, stop=True,
                )
            pfs = wpf.tile([P, D], f32, tag="pfsb")
            if t % 2 == 0:
                nc.vector.tensor_copy(pfs[:], pf[:])
            else:
                nc.scalar.activation(pfs[:], pf[:], AF.Copy)
            nc.sync.dma_start(out=Pp[1 + t * P:1 + (t + 1) * P, :], in_=pfs[:])
        close_pool("wpf")
        close_pool("ps_pf")
        close_pool("pcc")

        # ---- read back e (+ shifted), counts, gather offsets ----
        eF2 = sp.tile([P, TF], i32)
        nc.sync.dma_start(out=eF2[:], in_=e_buf[1:L + 1, :].rearrange("(p t) o -> p (t o)", p=P))
        eSH = sp.tile([P, TF], i32)
        nc.sync.dma_start(out=eSH[:], in_=e_buf[0:L, :].rearrange("(p t) o -> p (t o)", p=P))
        ef = sp.tile([P, TF], f32)
        nc.vector.tensor_copy(ef[:], eF2[:])
        efs = sp.tile([P, TF], f32)
        nc.vector.tensor_copy(efs[:], eSH[:])

        # 17-column gather offsets: col 0 = e[j-1]+1 (sentinel -> 0), cols 1..16 = e[j]+1
        gb17 = sp.tile([P, TF + 1], f32)
        nc.vector.tensor_copy(gb17[:, 0:1], efs[:, 0:1])
        nc.vector.tensor_copy(gb17[:, 1:TF + 1], ef[:])
        nc.vector.tensor_scalar(gb17[:], gb17[:], 1.0, None, A.add)
        gbi = sp.tile([P, TF + 1], i32)
        nc.vector.tensor_copy(gbi[:], gb17[:])

        cnt = sp.tile([P, TF], f32)
        nc.vector.tensor_tensor(cnt[:], ef[:], efs[:], A.subtract)
        rcnt = sp.tile([P, TF], f32)
        nc.vector.tensor_scalar(rcnt[:], cnt[:], 1e-9, None, A.add)
        nc.vector.reciprocal(rcnt[:], rcnt[:])
        valid = sp.tile([P, TF], f32)
        nc.vector.tensor_scalar(valid[:], liotaf_sb[:], tot_sb[:, 0:1], None, A.is_lt)
        nc.vector.tensor_tensor(rcnt[:], rcnt[:], valid[:], A.mult)

        # ---- loss (Stirling lgamma) from allreduced scalars ----
        nbv = scg[0:1, 0:1]
        tpv = scg[0:1, 1:2]
        x3 = sp.tile([1, 3], f32)
        nc.vector.tensor_scalar(x3[0:1, 0:1], tpv, 1.0, None, A.add)
        nc.vector.tensor_scalar(x3[0:1, 1:2], nbv, 1.0, None, A.add)
        nc.vector.tensor_tensor(x3[0:1, 2:3], tpv, nbv, A.subtract)
        nc.vector.tensor_scalar(x3[0:1, 2:3], x3[0:1, 2:3], 1.0, None, A.add)
        ln3 = sp.tile([1, 3], f32)
        nc.scalar.activation(ln3[:], x3[:], AF.Ln)
        lg = sp.tile([1, 3], f32)
        nc.vector.tensor_scalar(lg[:], x3[:], -0.5, None, A.add)
        nc.vector.tensor_tensor(lg[:], lg[:], ln3[:], A.mult)
        nc.vector.tensor_tensor(lg[:], lg[:], x3[:], A.subtract)
        nc.vector.tensor_scalar(lg[:], lg[:], HALF_LN_2PI, None, A.add)
        t12 = sp.tile([1, 3], f32)
        nc.vector.tensor_scalar(t12[:], x3[:], 12.0, None, A.mult)
        nc.vector.reciprocal(t12[:], t12[:])
        nc.vector.tensor_tensor(lg[:], lg[:], t12[:], A.add)
        x3c = sp.tile([1, 3], f32)
        nc.vector.tensor_tensor(x3c[:], x3[:], x3[:], A.mult)
        nc.vector.tensor_tensor(x3c[:], x3c[:], x3[:], A.mult)
        nc.vector.tensor_scalar(x3c[:], x3c[:], 360.0, None, A.mult)
        nc.vector.reciprocal(x3c[:], x3c[:])
        nc.vector.tensor_tensor(lg[:], lg[:], x3c[:], A.subtract)

        lpm = sp.tile([1, 1], f32)
        nc.vector.tensor_tensor(lpm[:], lg[0:1, 0:1], lg[0:1, 1:2], A.subtract)
        nc.vector.tensor_tensor(lpm[:], lpm[:], lg[0:1, 2:3], A.subtract)
        w1 = sp.tile([1, 1], f32)
        nc.vector.tensor_scalar(w1[:], nbv, LOG02, None, A.mult)
        nc.vector.tensor_tensor(lpm[:], lpm[:], w1[:], A.add)
        w2 = sp.tile([1, 1], f32)
        nc.vector.tensor_tensor(w2[:], tpv, nbv, A.subtract)
        nc.vector.tensor_scalar(w2[:], w2[:], LOG08, None, A.mult)
        nc.vector.tensor_tensor(lpm[:], lpm[:], w2[:], A.add)
        rtp = sp.tile([1, 1], f32)
        nc.vector.reciprocal(rtp[:], tpv)
        nc.vector.tensor_tensor(lpm[:], lpm[:], rtp[:], A.mult)
        nc.vector.tensor_scalar(lpm[:], lpm[:], -1.0, None, A.mult)

        nc.sync.dma_start(out=loss_d[:], in_=lpm[:])
        nc.sync.dma_start(out=nb_d[:], in_=nbv)
        nc.sync.dma_start(out=tp_d[:], in_=tpv)
        close_pool("ph")
        close_pool("pm")

        # ---- gather prefix rows: G[p, 1+t, :] = Pp[e[16p+t]+1], G[p, 0] = Pp[e[j-1]+1] ----
        pg = open_pool("pg")
        G = pg.tile([P, TF + 1, D], f32, tag="G")
        for t in range(TF + 1):
            nc.gpsimd.indirect_dma_start(
                out=G[:, t, :], out_offset=None,
                in_=Pp[:], in_offset=bass.IndirectOffsetOnAxis(ap=gbi[:, t:t + 1], axis=0),
            )

        # ---- pooled[:, t] = (G[t+1] - G[t]) * rcnt[t], in place (descending t) ----
        for t in range(TF - 1, -1, -1):
            nc.vector.tensor_tensor(G[:, t + 1, :], G[:, t + 1, :], G[:, t, :], A.subtract)
        for t in range(TF):
            nc.vector.tensor_scalar(G[:, t + 1, :], G[:, t + 1, :], rcnt[:, t:t + 1], None, A.mult)
        nc.sync.dma_start(out=pooled_d.rearrange("(p t) d -> p t d", p=P), in_=G[:, 1:TF + 1, :])
        close_pool("pg")

    nc.compile()
    return nc


def _get_built():
    global _BUILT
    if _BUILT is None:
        _BUILT = _build()
    return _BUILT


def kernel(hidden, attention_mask, noise, Wq, Wk):
    from concourse.bass_utils import run_bass_kernel_spmd

    nc = _get_built()
    B = hidden.shape[0]
    assert B == NC
    consts = _consts()
    hidden = np.ascontiguousarray(hidden, np.float32)
    Wq = np.ascontiguousarray(Wq, np.float32)
    Wk = np.ascontiguousarray(Wk, np.float32)
    in_maps = []
    for c in range(NC):
        m = dict(consts)
        m["h"] = hidden[c]
        m["Wq"] = Wq
        m["Wk"] = Wk
        m["mask_f2"] = np.ascontiguousarray(attention_mask[c].reshape(P, TF), np.int32)
        m["noise_f2"] = np.ascontiguousarray(noise[c].reshape(P, TF), np.float32)
        in_maps.append(m)
    res = run_bass_kernel_spmd(nc, in_maps, list(range(NC))).results

    pooled = np.stack([res[c]["pooled"] for c in range(NC)])
    pmask = np.stack([res[c]["pmask"].reshape(L) for c in range(NC)]).astype(np.int32)
    loss = np.float32(res[0]["loss"].reshape(())[()])
    nb = np.float32(res[0]["nb"].reshape(())[()])
    tp = np.float32(res[0]["tp"].reshape(())[()])
    return pooled, pmask, loss, nb, tp
